# revision 6
# baseline (speedup 1.0000x reference)
"""Multi-head attention (B=2, S=4096, E=512, H=8) on 8 trn2 NeuronCores.

Sharding: data-parallel over B (cores 0-3 -> b=0, 4-7 -> b=1) and
sequence-parallel over the query dim (each core owns a 1024-query chunk,
all 8 heads).

Per core:
  - Q/K projections in fp8e4m3 with DoubleRow perf mode (contract 256 per
    instruction at 0.5 cycles/row): query/keys/Wq/Wk are shipped pre-cast
    to fp8 by the host.
  - Projection PSUM is copy-cast (DVE) to fp8 staging, then fixup DMAs
    rearrange into the DoubleRow slab layout kT8/qT8 [128, 2, 2, S]:
    head h = 2c+s lives on partitions 32c..32c+32, sub-slot s, slab
    j = d % 2, pp = d // 2 (the d=2pp+j interleave makes the PSUM->slab
    DMA a single contiguous-stream copy per (chunk, c, s)).
  - Scores: one DoubleRow matmul per (head, kt): lhsT=kT8 [32,2,128],
    rhs=qT8 [32,2,512] -> PSUM [128, 512], 256 cycles.
  - p = exp(scores/sqrt(E)) on the scalar engine (no max-subtraction:
    |logits| < ~1), then multiplied by the 0/1 mask on DVE (most kt) or
    Pool/gpsimd (POOL_KT set) to balance engines.
  - V projection stays bf16 (fp8 V would cost ~2.5% output error);
    interleaved into the first attention pass (c=0, qt=0) so the scalar
    engine starts early.  PV matmul in bf16 with a ones-column in V
    (row 64 = softmax denominator).
  - Normalize: denominator broadcast via DRAM bounce, then a single
    tensor_tensor divide.
  - Output projection per head (contract 64) + bias rank-1 matmul.
"""

import math

import ml_dtypes
import numpy as np

B, S, E, H = 2, 4096, 512, 8
HD = E // H  # 64
P = 128
NCORES = 8
QC = (B * S) // NCORES  # 1024 queries per core
NKT = S // P            # 32 k-subtiles of 128
NQT = QC // 512         # 2 q-tiles of 512
NPAIR = H // 2          # 4 head pairs
SCALE = 1.0 / math.sqrt(E)
BF16 = ml_dtypes.bfloat16
FP8 = ml_dtypes.float8_e4m3fn

# kt indices whose mask-multiplies run on gpsimd (Pool) instead of DVE
POOL_KT = frozenset((3, 7, 11, 15, 19, 23, 27, 31))

_CACHE = {}
LAST_RESULT = None  # BassKernelResults of the most recent run (for test.py)


def _build():
    if "nc" in _CACHE:
        return _CACHE["nc"]

    import concourse.bass as bass
    import concourse.tile as tile
    from concourse import bacc, mybir

    f32 = mybir.dt.float32
    bf16 = mybir.dt.bfloat16
    fp8 = mybir.dt.float8e4
    DR = mybir.MatmulPerfMode.DoubleRow
    Exp = mybir.ActivationFunctionType.Exp

    nc = bacc.Bacc(
        "TRN2", target_bir_lowering=False, debug=False, num_devices=NCORES
    )

    maskT = nc.dram_tensor("maskT", [S, QC], bf16, kind="ExternalInput").ap()
    keys8 = nc.dram_tensor("keys8", [P, 4, S], fp8, kind="ExternalInput").ap()
    valsT = nc.dram_tensor("valsT", [E, S], bf16, kind="ExternalInput").ap()
    qry8 = nc.dram_tensor("qry8", [P, 4, QC], fp8, kind="ExternalInput").ap()
    wq8 = nc.dram_tensor("wq8", [P, 4, E], fp8, kind="ExternalInput").ap()
    wk8 = nc.dram_tensor("wk8", [P, 4, E], fp8, kind="ExternalInput").ap()
    wvT = nc.dram_tensor("wvT", [E, E], bf16, kind="ExternalInput").ap()
    woT = nc.dram_tensor("woT", [E, E], bf16, kind="ExternalInput").ap()
    bo = nc.dram_tensor("bo", [E], f32, kind="ExternalInput").ap()
    out = nc.dram_tensor("out", [QC, E], f32, kind="ExternalOutput").ap()

    with tile.TileContext(nc) as tc:
        with tc.tile_pool(name="persist", bufs=1) as persist:
            # persistent SBUF tensors (per-partition bytes in comments)
            maskb = persist.tile([P, NKT, QC], bf16)          # 64 KB
            v_all = persist.tile([P, NKT, H, HD + 1], bf16)   # 33.3 KB
            # DoubleRow slab layout, head h=2c+s on partitions 32c..32c+32.
            # Matmul APs only allow base partitions {0, 32, 64}, so pairs
            # 0-2 share one tile (bases 0/32/64) and pair 3 gets its own
            # tile at base 0.
            kT8a = persist.tile([96, 2, 2, S], fp8)           # 16 KB
            kT8b = persist.tile([32, 2, 2, S], fp8)           # 16 KB
            qT8a = persist.tile([96, 2, 2, QC], fp8)          # 4 KB
            qT8b = persist.tile([32, 2, 2, QC], fp8)          # 4 KB

            def kT8(c):
                return kT8b if c == 3 else kT8a[32 * c : 32 * (c + 1)]

            def qT8(c):
                return qT8b if c == 3 else qT8a[32 * c : 32 * (c + 1)]
            attn_all = persist.tile([HD, H, QC], bf16)        # 16 KB
            wq_sb = persist.tile([P, 4, E], fp8)              # 2 KB
            wk_sb = persist.tile([P, 4, E], fp8)              # 2 KB
            wv_sb = persist.tile([P, 4, E], bf16)             # 4 KB
            wo_sb = persist.tile([HD, H, E], bf16)            # 8 KB
            qry_sb = persist.tile([P, 4, QC], fp8)            # 4 KB
            bo_sb = persist.tile([1, E], f32)

            nc.sync.dma_start(out=wq_sb, in_=wq8)
            nc.sync.dma_start(out=wk_sb, in_=wk8)
            nc.sync.dma_start(
                out=wv_sb, in_=wvT.rearrange("(c p) o -> p c o", p=P)
            )
            nc.sync.dma_start(
                out=wo_sb, in_=woT.rearrange("(h d) o -> d h o", d=HD)
            )
            nc.sync.dma_start(out=bo_sb, in_=bo[None, :])
            nc.sync.dma_start(out=qry_sb, in_=qry8)

            # mask: bf16 0/1, [k-part, kt, q]
            for kt in range(NKT):
                nc.sync.dma_start(
                    out=maskb[:, kt, :],
                    in_=maskT[kt * P : (kt + 1) * P, :],
                )

            # ones column for the softmax denominator
            nc.vector.memset(v_all[:, :, :, HD : HD + 1], 1.0)

            # ---- Q projection: fp8 DoubleRow ----
            with (
                tc.tile_pool(name="qkstage", bufs=2) as qkstage,
                tc.tile_pool(name="qkps", bufs=2, space="PSUM") as qkps,
            ):
                for qt in range(NQT):
                    qf8c = qkstage.tile([P, 4, 512], fp8, tag="qf8")
                    for c in range(NPAIR):
                        ps = qkps.tile([P, 512], f32)
                        for e2 in range(2):
                            nc.tensor.matmul(
                                ps,
                                lhsT=wq_sb[:, 2 * e2 : 2 * e2 + 2,
                                           c * P : (c + 1) * P],
                                rhs=qry_sb[:, 2 * e2 : 2 * e2 + 2,
                                           qt * 512 : (qt + 1) * 512],
                                start=(e2 == 0),
                                stop=(e2 == 1),
                                perf_mode=DR,
                            )
                        nc.vector.tensor_copy(out=qf8c[:, c, :], in_=ps)
                    # fixup DMAs: psum-layout [s*64+d, c, q] -> slab layout
                    # qT8[32c+pp, s, j, q] with d = 2*pp + j
                    for c in range(NPAIR):
                        for s_ in range(2):
                            nc.sync.dma_start(
                                out=qT8(c)[:, s_, :,
                                           qt * 512 : (qt + 1) * 512],
                                in_=qf8c[s_ * HD : (s_ + 1) * HD, c, :],
                            )

                # ---- K projection: fp8 DoubleRow ----
                for kc in range(S // 512):
                    ksl = slice(kc * 512, (kc + 1) * 512)
                    k8c = qkstage.tile([P, 4, 512], fp8, tag="k8c")
                    nc.sync.dma_start(out=k8c, in_=keys8[:, :, ksl])
                    kf8c = qkstage.tile([P, 4, 512], fp8, tag="kf8")
                    for c in range(NPAIR):
                        ps = qkps.tile([P, 512], f32)
                        for e2 in range(2):
                            nc.tensor.matmul(
                                ps,
                                lhsT=wk_sb[:, 2 * e2 : 2 * e2 + 2,
                                           c * P : (c + 1) * P],
                                rhs=k8c[:, 2 * e2 : 2 * e2 + 2, :],
                                start=(e2 == 0),
                                stop=(e2 == 1),
                                perf_mode=DR,
                            )
                        nc.vector.tensor_copy(out=kf8c[:, c, :], in_=ps)
                    for c in range(NPAIR):
                        for s_ in range(2):
                            nc.sync.dma_start(
                                out=kT8(c)[:, s_, :, ksl],
                                in_=kf8c[s_ * HD : (s_ + 1) * HD, c, :],
                            )

            # ---- attention (V projection interleaved into c=0, qt=0) ----
            with (
                tc.tile_pool(name="vstage", bufs=2) as vstage,
                tc.tile_pool(name="vps", bufs=2, space="PSUM") as vps,
                tc.tile_pool(name="scps", bufs=2, space="PSUM") as scps,
                tc.tile_pool(name="pvps", bufs=2, space="PSUM") as pvps,
                tc.tile_pool(name="pp", bufs=3) as pp,
                tc.tile_pool(name="norm", bufs=2) as norm,
                tc.tile_pool(name="ndram", bufs=2, space="DRAM") as ndram,
            ):
                for c in range(NPAIR):
                    for qt in range(NQT):
                        qsl = slice(qt * 512, (qt + 1) * 512)
                        pv0 = pvps.tile([HD + 1, 512], f32, tag="pv")
                        pv1 = pvps.tile([HD + 1, 512], f32, tag="pv")
                        for kt in range(NKT):
                            if c == 0 and qt == 0:
                                # V projection for this kt (bf16)
                                vs = vstage.tile([P, 4, P], bf16)
                                nc.sync.dma_start(
                                    out=vs,
                                    in_=valsT[:, kt * P : (kt + 1) * P]
                                    .rearrange("(g p) s -> p g s", p=P),
                                )
                                vp = vps.tile([P, E], f32)
                                for ec in range(4):
                                    nc.tensor.matmul(
                                        vp,
                                        lhsT=vs[:, ec, :],
                                        rhs=wv_sb[:, ec, :],
                                        start=(ec == 0),
                                        stop=(ec == 3),
                                    )
                                nc.vector.tensor_copy(
                                    out=v_all[:, kt, :, 0:HD],
                                    in_=vp.rearrange("p (h d) -> p h d", h=H),
                                )
                            ksl = slice(kt * P, (kt + 1) * P)
                            sc = scps.tile([P, 2, 512], f32)
                            for s_ in range(2):
                                nc.tensor.matmul(
                                    sc[:, s_, :],
                                    lhsT=kT8(c)[:, s_, :, ksl],
                                    rhs=qT8(c)[:, s_, :, qsl],
                                    start=True,
                                    stop=True,
                                    perf_mode=DR,
                                )
                            p_sb = pp.tile([P, 2, 512], bf16)
                            nc.scalar.activation(p_sb, sc, Exp, scale=SCALE)
                            meng = (
                                nc.gpsimd if kt in POOL_KT else nc.vector
                            )
                            for s_ in range(2):
                                meng.tensor_tensor(
                                    out=p_sb[:, s_, :],
                                    in0=p_sb[:, s_, :],
                                    in1=maskb[:, kt, qsl],
                                    op=mybir.AluOpType.mult,
                                )
                            nc.tensor.matmul(
                                pv0,
                                lhsT=v_all[:, kt, 2 * c, :],
                                rhs=p_sb[:, 0, :],
                                start=(kt == 0),
                                stop=(kt == NKT - 1),
                            )
                            nc.tensor.matmul(
                                pv1,
                                lhsT=v_all[:, kt, 2 * c + 1, :],
                                rhs=p_sb[:, 1, :],
                                start=(kt == 0),
                                stop=(kt == NKT - 1),
                            )
                        for s_, pv in ((0, pv0), (1, pv1)):
                            h = 2 * c + s_
                            # copy PV out of PSUM right away (frees the bank
                            # for the next (c, qt) iteration's accumulation)
                            pv_sb = norm.tile([P, 512], f32, tag="den")
                            nc.vector.tensor_copy(
                                out=pv_sb[0 : HD + 1, :],
                                in_=pv[0 : HD + 1, :],
                            )
                            # replicate den across partitions 0..63 via a
                            # DRAM bounce (DRAM sources allow stride-0
                            # partition broadcast APs; SBUF sources don't)
                            dscr = ndram.tile([1, 512], f32, tag="dscr")
                            nc.sync.dma_start(
                                out=dscr, in_=pv_sb[HD : HD + 1, :]
                            )
                            den_rep = norm.tile([HD, 512], f32, tag="denr")
                            nc.sync.dma_start(
                                out=den_rep,
                                in_=bass.AP(
                                    tensor=dscr.tensor,
                                    offset=dscr.offset,
                                    ap=[[0, HD], [1, 512]],
                                ),
                            )
                            rep_sb = norm.tile([HD, 512], f32, tag="rep")
                            nc.vector.reciprocal_approx_fast(
                                out=rep_sb, in_=den_rep
                            )
                            nc.vector.tensor_tensor(
                                out=attn_all[:, h, qsl],
                                in0=pv_sb[0:HD, :],
                                in1=rep_sb,
                                op=mybir.AluOpType.mult,
                            )

            # ---- output projection + bias ----
            with (
                tc.tile_pool(name="ops", bufs=2, space="PSUM") as ops,
                tc.tile_pool(name="osb", bufs=3) as osb,
                tc.tile_pool(name="onesp", bufs=1) as onesp,
            ):
                ones1 = onesp.tile([1, P], f32)
                nc.vector.memset(ones1, 1.0)
                for q8 in range(QC // P):
                    ps = ops.tile([P, E], f32)
                    for h in range(H):
                        nc.tensor.matmul(
                            ps,
                            lhsT=attn_all[:, h, q8 * P : (q8 + 1) * P],
                            rhs=wo_sb[:, h, :],
                            start=(h == 0),
                            stop=False,
                        )
                    # bias via rank-1 matmul: ones^T (1x128) @ bo (1x512)
                    nc.tensor.matmul(
                        ps,
                        lhsT=ones1,
                        rhs=bo_sb,
                        start=False,
                        stop=True,
                    )
                    ob = osb.tile([P, E], f32)
                    nc.vector.tensor_copy(out=ob, in_=ps)
                    nc.sync.dma_start(
                        out=out[q8 * P : (q8 + 1) * P, :], in_=ob
                    )

    nc.compile()
    _CACHE["nc"] = nc
    return nc


def _slab(wT):
    """[E_in, X] -> [128, 4, X] with in-dim e = g*128 + p, cast fp8."""
    return np.ascontiguousarray(
        wT.reshape(4, P, -1).transpose(1, 0, 2).astype(FP8)
    )


def make_in_maps(values, keys, query, mask, Wv, Wk, Wq, Wo, bo):
    values = np.asarray(values, np.float32)
    keys = np.asarray(keys, np.float32)
    query = np.asarray(query, np.float32)
    mask = np.asarray(mask)
    wq8 = _slab(np.asarray(Wq, np.float32).T)
    wk8 = _slab(np.asarray(Wk, np.float32).T)
    wvT = np.ascontiguousarray(np.asarray(Wv, np.float32).T.astype(BF16))
    woT = np.ascontiguousarray(np.asarray(Wo, np.float32).T.astype(BF16))
    bo = np.ascontiguousarray(np.asarray(bo, np.float32))

    in_maps = []
    for core in range(NCORES):
        b, qc = core // (NCORES // B), core % (NCORES // B)
        qsl = slice(qc * QC, (qc + 1) * QC)
        in_maps.append(
            {
                "maskT": np.ascontiguousarray(
                    mask[b, 0, qsl, :].T.astype(BF16)
                ),
                "keys8": _slab(keys[b].T),
                "valsT": np.ascontiguousarray(values[b].T.astype(BF16)),
                "qry8": _slab(query[b, qsl].T),
                "wq8": wq8,
                "wk8": wk8,
                "wvT": wvT,
                "woT": woT,
                "bo": bo,
            }
        )
    return in_maps


def kernel(values, keys, query, mask, Wv, Wk, Wq, Wo, bo):
    global LAST_RESULT
    from concourse.bass_utils import run_bass_kernel_spmd

    nc = _build()
    in_maps = make_in_maps(values, keys, query, mask, Wv, Wk, Wq, Wo, bo)
    res = run_bass_kernel_spmd(nc, in_maps, core_ids=list(range(NCORES)))
    LAST_RESULT = res

    out = np.empty((B, S, E), np.float32)
    for core in range(NCORES):
        b, qc = core // (NCORES // B), core % (NCORES // B)
        out[b, qc * QC : (qc + 1) * QC] = res.results[core]["out"]
    return out


# revision 8
# speedup vs baseline: 1.1238x; 1.1238x over previous
"""Multi-head attention (B=2, S=4096, E=512, H=8) on 8 trn2 NeuronCores.

Sharding: data-parallel over B (cores 0-3 -> b=0, 4-7 -> b=1) and
sequence-parallel over the query dim (each core owns a 1024-query chunk,
all 8 heads).

Per core (all matmuls bf16; the PE streams ~1 output column/cycle, so the
structure minimizes total streamed columns and keeps the PE queue full to
hold the high p-state):
  - Q projection up front; K projection for pair c emitted at the top of
    pair c's attention loop (overlaps the previous pair's attention);
    V projection interleaved into the first attention pass (c=0, qt=0) so
    the scalar engine starts within a few microseconds.
  - scores^T = k_h q_h^T (contract 64, k on partitions, two heads per
    PSUM tile), p = exp(scores/sqrt(E)) on the scalar engine (no
    max-subtraction: |logits| < ~1), then multiplied by the 0/1 mask on
    DVE (most kt) or gpsimd/Pool (POOL_KT) to balance engines.
  - PV via matmul with a ones-column appended to V (row 64 = softmax
    denominator); normalize with a reciprocal broadcast via DRAM bounce.
  - Output projection packs two heads per matmul (contract 128): attn is
    stored as attn2 [128, pair, q] with odd heads DMA-shifted to
    partitions 64..127; bias via rank-1 matmul.

All DMAs ride the sync (SP) hardware DGE queue; PSUM->SBUF copies and the
normalize chain are on DVE.
"""

import math

import ml_dtypes
import numpy as np

B, S, E, H = 2, 4096, 512, 8
HD = E // H  # 64
P = 128
NCORES = 8
QC = (B * S) // NCORES  # 1024 queries per core
NKT = S // P            # 32 k-subtiles of 128
NQT = QC // 512         # 2 q-tiles of 512
NPAIR = H // 2          # 4 head pairs
SCALE = 1.0 / math.sqrt(E)
BF16 = ml_dtypes.bfloat16

# kt indices whose mask-multiplies run on gpsimd (Pool) instead of DVE
POOL_KT = frozenset((2, 5, 8, 11, 14, 17, 20, 23, 26, 29))

_CACHE = {}
LAST_RESULT = None  # BassKernelResults of the most recent run (for test.py)


def _build():
    if "nc" in _CACHE:
        return _CACHE["nc"]

    import concourse.bass as bass
    import concourse.tile as tile
    from concourse import bacc, mybir

    f32 = mybir.dt.float32
    bf16 = mybir.dt.bfloat16
    Exp = mybir.ActivationFunctionType.Exp

    nc = bacc.Bacc(
        "TRN2", target_bir_lowering=False, debug=False, num_devices=NCORES
    )

    maskT = nc.dram_tensor("maskT", [S, QC], bf16, kind="ExternalInput").ap()
    keysT = nc.dram_tensor("keysT", [E, S], bf16, kind="ExternalInput").ap()
    valsT = nc.dram_tensor("valsT", [E, S], bf16, kind="ExternalInput").ap()
    qryT = nc.dram_tensor("qryT", [E, QC], bf16, kind="ExternalInput").ap()
    wqT = nc.dram_tensor("wqT", [E, E], bf16, kind="ExternalInput").ap()
    wkT = nc.dram_tensor("wkT", [E, E], bf16, kind="ExternalInput").ap()
    wvT = nc.dram_tensor("wvT", [E, E], bf16, kind="ExternalInput").ap()
    wo2d = nc.dram_tensor("wo2d", [P, NPAIR, E], bf16, kind="ExternalInput").ap()
    bo = nc.dram_tensor("bo", [E], f32, kind="ExternalInput").ap()
    out = nc.dram_tensor("out", [QC, E], f32, kind="ExternalOutput").ap()

    with tile.TileContext(nc) as tc:
        with tc.tile_pool(name="persist", bufs=1) as persist:
            # persistent SBUF tensors (per-partition bytes in comments)
            maskb = persist.tile([P, NKT, QC], bf16)         # 64 KB
            v_all = persist.tile([P, NKT, H, HD + 1], bf16)  # 33.3 KB
            kT_all = persist.tile([P, NPAIR, S], bf16)       # 32 KB
            qT_all = persist.tile([P, NPAIR, QC], bf16)      # 8 KB
            attn2 = persist.tile([P, NPAIR, QC], bf16)       # 8 KB
            wq_sb = persist.tile([P, 4, E], bf16)            # 4 KB
            wk_sb = persist.tile([P, 4, E], bf16)            # 4 KB
            wv_sb = persist.tile([P, 4, E], bf16)            # 4 KB
            wo_sb = persist.tile([P, NPAIR, E], bf16)        # 4 KB
            qry_sb = persist.tile([P, 4, QC], bf16)          # 8 KB
            bo_sb = persist.tile([1, E], f32)

            nc.sync.dma_start(
                out=wq_sb, in_=wqT.rearrange("(g p) o -> p g o", p=P)
            )
            nc.sync.dma_start(
                out=wk_sb, in_=wkT.rearrange("(g p) o -> p g o", p=P)
            )
            nc.sync.dma_start(
                out=wv_sb, in_=wvT.rearrange("(g p) o -> p g o", p=P)
            )
            nc.sync.dma_start(out=wo_sb, in_=wo2d)
            nc.sync.dma_start(out=bo_sb, in_=bo[None, :])
            nc.sync.dma_start(
                out=qry_sb, in_=qryT.rearrange("(g p) q -> p g q", p=P)
            )

            # mask: bf16 0/1, [k-part, kt, q]
            for kt in range(NKT):
                nc.sync.dma_start(
                    out=maskb[:, kt, :],
                    in_=maskT[kt * P : (kt + 1) * P, :],
                )

            # ones column for the softmax denominator
            nc.vector.memset(v_all[:, :, :, HD : HD + 1], 1.0)

            # ---- Q projection (up front, cheap: 16k columns) ----
            with tc.tile_pool(name="qps", bufs=2, space="PSUM") as qps:
                for qt in range(NQT):
                    qsl = slice(qt * 512, (qt + 1) * 512)
                    for c in range(NPAIR):
                        ps = qps.tile([P, 512], f32)
                        for ec in range(4):
                            nc.tensor.matmul(
                                ps,
                                lhsT=wq_sb[:, ec, c * P : (c + 1) * P],
                                rhs=qry_sb[:, ec, qsl],
                                start=(ec == 0),
                                stop=(ec == 3),
                            )
                        nc.vector.tensor_copy(out=qT_all[:, c, qsl], in_=ps)

            # ---- attention (K proj per pair, V proj inside c0/qt0) ----
            with (
                tc.tile_pool(name="kstage", bufs=2) as kstage,
                tc.tile_pool(name="vstage", bufs=2) as vstage,
                tc.tile_pool(name="kps", bufs=1, space="PSUM") as kps,
                tc.tile_pool(name="vps", bufs=1, space="PSUM") as vps,
                tc.tile_pool(name="scps", bufs=2, space="PSUM") as scps,
                tc.tile_pool(name="pvps", bufs=2, space="PSUM") as pvps,
                tc.tile_pool(name="pp", bufs=3) as pp,
                tc.tile_pool(name="norm", bufs=2) as norm,
                tc.tile_pool(name="ndram", bufs=2, space="DRAM") as ndram,
            ):
                for c in range(NPAIR):
                    # project kT for this pair (overlaps prev pair's attn)
                    for kc in range(S // 512):
                        ksl = slice(kc * 512, (kc + 1) * 512)
                        ks = kstage.tile([P, 4, 512], bf16)
                        nc.sync.dma_start(
                            out=ks,
                            in_=keysT[:, ksl].rearrange(
                                "(g p) s -> p g s", p=P
                            ),
                        )
                        ps = kps.tile([P, 512], f32)
                        for ec in range(4):
                            nc.tensor.matmul(
                                ps,
                                lhsT=wk_sb[:, ec, c * P : (c + 1) * P],
                                rhs=ks[:, ec, :],
                                start=(ec == 0),
                                stop=(ec == 3),
                            )
                        nc.vector.tensor_copy(out=kT_all[:, c, ksl], in_=ps)
                    for qt in range(NQT):
                        qsl = slice(qt * 512, (qt + 1) * 512)
                        pv0 = pvps.tile([HD + 1, 512], f32, tag="pv")
                        pv1 = pvps.tile([HD + 1, 512], f32, tag="pv")
                        for kt in range(NKT):
                            if c == 0 and qt == 0:
                                # V projection for this kt
                                vs = vstage.tile([P, 4, P], bf16)
                                nc.sync.dma_start(
                                    out=vs,
                                    in_=valsT[:, kt * P : (kt + 1) * P]
                                    .rearrange("(g p) s -> p g s", p=P),
                                )
                                vp = vps.tile([P, E], f32)
                                for ec in range(4):
                                    nc.tensor.matmul(
                                        vp,
                                        lhsT=vs[:, ec, :],
                                        rhs=wv_sb[:, ec, :],
                                        start=(ec == 0),
                                        stop=(ec == 3),
                                    )
                                nc.vector.tensor_copy(
                                    out=v_all[:, kt, :, 0:HD],
                                    in_=vp.rearrange("p (h d) -> p h d", h=H),
                                )
                            ksl = slice(kt * P, (kt + 1) * P)
                            sc = scps.tile([P, 2, 512], f32, tag="sc")
                            nc.tensor.matmul(
                                sc[:, 0, :],
                                lhsT=kT_all[0:HD, c, ksl],
                                rhs=qT_all[0:HD, c, qsl],
                                start=True,
                                stop=True,
                            )
                            nc.tensor.matmul(
                                sc[:, 1, :],
                                lhsT=kT_all[HD : 2 * HD, c, ksl],
                                rhs=qT_all[HD : 2 * HD, c, qsl],
                                start=True,
                                stop=True,
                            )
                            p_sb = pp.tile([P, 2, 512], bf16)
                            nc.scalar.activation(p_sb, sc, Exp, scale=SCALE)
                            meng = nc.gpsimd if kt in POOL_KT else nc.vector
                            for s_ in range(2):
                                meng.tensor_tensor(
                                    out=p_sb[:, s_, :],
                                    in0=p_sb[:, s_, :],
                                    in1=maskb[:, kt, qsl],
                                    op=mybir.AluOpType.mult,
                                )
                            nc.tensor.matmul(
                                pv0,
                                lhsT=v_all[:, kt, 2 * c, :],
                                rhs=p_sb[:, 0, :],
                                start=(kt == 0),
                                stop=(kt == NKT - 1),
                            )
                            nc.tensor.matmul(
                                pv1,
                                lhsT=v_all[:, kt, 2 * c + 1, :],
                                rhs=p_sb[:, 1, :],
                                start=(kt == 0),
                                stop=(kt == NKT - 1),
                            )
                        for s_, pv in ((0, pv0), (1, pv1)):
                            # copy PV out of PSUM right away (frees the bank
                            # for the next (c, qt) iteration's accumulation)
                            pv_sb = norm.tile([P, 512], f32, tag="den")
                            nc.vector.tensor_copy(
                                out=pv_sb[0 : HD + 1, :],
                                in_=pv[0 : HD + 1, :],
                            )
                            # replicate den across partitions 0..63 via a
                            # DRAM bounce (DRAM sources allow stride-0
                            # partition broadcast APs; SBUF sources don't),
                            # then reciprocal at base partition 0
                            dscr = ndram.tile([1, 512], f32, tag="dscr")
                            nc.sync.dma_start(
                                out=dscr, in_=pv_sb[HD : HD + 1, :]
                            )
                            den_rep = norm.tile([HD, 512], f32, tag="denr")
                            nc.sync.dma_start(
                                out=den_rep,
                                in_=bass.AP(
                                    tensor=dscr.tensor,
                                    offset=dscr.offset,
                                    ap=[[0, HD], [1, 512]],
                                ),
                            )
                            rep_sb = norm.tile([HD, 512], f32, tag="rep")
                            nc.vector.reciprocal_approx_fast(
                                out=rep_sb, in_=den_rep
                            )
                            if s_ == 0:
                                nc.vector.tensor_tensor(
                                    out=attn2[0:HD, c, qsl],
                                    in0=pv_sb[0:HD, :],
                                    in1=rep_sb,
                                    op=mybir.AluOpType.mult,
                                )
                            else:
                                # odd head: normalize into a temp, then
                                # DMA-shift to partitions 64..127 so the
                                # output projection can pack the pair
                                # (contract 128)
                                atmp = norm.tile([HD, 512], bf16, tag="atm")
                                nc.vector.tensor_tensor(
                                    out=atmp,
                                    in0=pv_sb[0:HD, :],
                                    in1=rep_sb,
                                    op=mybir.AluOpType.mult,
                                )
                                nc.sync.dma_start(
                                    out=attn2[HD : 2 * HD, c, qsl],
                                    in_=atmp,
                                )

            # ---- output projection (2 heads packed) + bias ----
            with (
                tc.tile_pool(name="ops", bufs=2, space="PSUM") as ops,
                tc.tile_pool(name="osb", bufs=3) as osb,
                tc.tile_pool(name="onesp", bufs=1) as onesp,
            ):
                ones1 = onesp.tile([1, P], f32)
                nc.vector.memset(ones1, 1.0)
                for q8 in range(QC // P):
                    ps = ops.tile([P, E], f32)
                    for c in range(NPAIR):
                        nc.tensor.matmul(
                            ps,
                            lhsT=attn2[:, c, q8 * P : (q8 + 1) * P],
                            rhs=wo_sb[:, c, :],
                            start=(c == 0),
                            stop=False,
                        )
                    # bias via rank-1 matmul: ones^T (1x128) @ bo (1x512)
                    nc.tensor.matmul(
                        ps,
                        lhsT=ones1,
                        rhs=bo_sb,
                        start=False,
                        stop=True,
                    )
                    ob = osb.tile([P, E], f32)
                    nc.vector.tensor_copy(out=ob, in_=ps)
                    nc.sync.dma_start(
                        out=out[q8 * P : (q8 + 1) * P, :], in_=ob
                    )

    nc.compile()
    _CACHE["nc"] = nc
    return nc


def make_in_maps(values, keys, query, mask, Wv, Wk, Wq, Wo, bo):
    values = np.asarray(values, np.float32)
    keys = np.asarray(keys, np.float32)
    query = np.asarray(query, np.float32)
    mask = np.asarray(mask)
    wqT = np.ascontiguousarray(np.asarray(Wq, np.float32).T.astype(BF16))
    wkT = np.ascontiguousarray(np.asarray(Wk, np.float32).T.astype(BF16))
    wvT = np.ascontiguousarray(np.asarray(Wv, np.float32).T.astype(BF16))
    # wo2d[s*64+d, c, e] = Wo[e, (2c+s)*64+d]
    wo2d = np.ascontiguousarray(
        np.asarray(Wo, np.float32).T.reshape(NPAIR, 2, HD, E)
        .transpose(1, 2, 0, 3).reshape(P, NPAIR, E).astype(BF16)
    )
    bo = np.ascontiguousarray(np.asarray(bo, np.float32))

    in_maps = []
    for core in range(NCORES):
        b, qc = core // (NCORES // B), core % (NCORES // B)
        qsl = slice(qc * QC, (qc + 1) * QC)
        in_maps.append(
            {
                "maskT": np.ascontiguousarray(
                    mask[b, 0, qsl, :].T.astype(BF16)
                ),
                "keysT": np.ascontiguousarray(keys[b].T.astype(BF16)),
                "valsT": np.ascontiguousarray(values[b].T.astype(BF16)),
                "qryT": np.ascontiguousarray(query[b, qsl].T.astype(BF16)),
                "wqT": wqT,
                "wkT": wkT,
                "wvT": wvT,
                "wo2d": wo2d,
                "bo": bo,
            }
        )
    return in_maps


def kernel(values, keys, query, mask, Wv, Wk, Wq, Wo, bo):
    global LAST_RESULT
    from concourse.bass_utils import run_bass_kernel_spmd

    nc = _build()
    in_maps = make_in_maps(values, keys, query, mask, Wv, Wk, Wq, Wo, bo)
    res = run_bass_kernel_spmd(nc, in_maps, core_ids=list(range(NCORES)))
    LAST_RESULT = res

    out = np.empty((B, S, E), np.float32)
    for core in range(NCORES):
        b, qc = core // (NCORES // B), core % (NCORES // B)
        out[b, qc * QC : (qc + 1) * QC] = res.results[core]["out"]
    return out


# revision 9
# speedup vs baseline: 1.6132x; 1.4354x over previous
"""Multi-head attention (B=2, S=4096, E=512, H=8) on 8 trn2 NeuronCores.

Sharding: data-parallel over B (cores 0-3 -> b=0, 4-7 -> b=1) and
sequence-parallel over the query dim (each core owns a 1024-query chunk,
all 8 heads).

Per core (all matmuls bf16; the PE streams ~1 output column/cycle, so the
structure minimizes total streamed columns and keeps the PE queue full to
hold the high p-state):
  - Q projection up front; K projection for pair c emitted at the top of
    pair c's attention loop (overlaps the previous pair's attention);
    V projection interleaved into the first attention pass (c=0, qt=0) so
    the scalar engine starts within a few microseconds.
  - scores^T = k_h q_h^T (contract 64, k on partitions, two heads per
    PSUM tile), p = exp(scores/sqrt(E)) on the scalar engine (no
    max-subtraction: |logits| < ~1), then multiplied by the 0/1 mask on
    DVE (most kt) or gpsimd/Pool (POOL_KT) to balance engines.
  - PV via matmul with a ones-column appended to V (row 64 = softmax
    denominator); normalize with a reciprocal broadcast via DRAM bounce.
  - Output projection packs two heads per matmul (contract 128): attn is
    stored as attn2 [128, pair, q] with odd heads DMA-shifted to
    partitions 64..127; bias via rank-1 matmul.

All DMAs ride the sync (SP) hardware DGE queue; PSUM->SBUF copies and the
normalize chain are on DVE.
"""

import math

import ml_dtypes
import numpy as np

B, S, E, H = 2, 4096, 512, 8
HD = E // H  # 64
P = 128
NCORES = 8
QC = (B * S) // NCORES  # 1024 queries per core
NKT = S // P            # 32 k-subtiles of 128
NQT = QC // 512         # 2 q-tiles of 512
NPAIR = H // 2          # 4 head pairs
SCALE = 1.0 / math.sqrt(E)
BF16 = ml_dtypes.bfloat16

# kt indices whose mask-multiplies run on gpsimd (Pool) instead of DVE
POOL_KT = frozenset()

_CACHE = {}
LAST_RESULT = None  # BassKernelResults of the most recent run (for test.py)


def _build():
    if "nc" in _CACHE:
        return _CACHE["nc"]

    import concourse.bass as bass
    import concourse.tile as tile
    from concourse import bacc, mybir

    f32 = mybir.dt.float32
    bf16 = mybir.dt.bfloat16
    Exp = mybir.ActivationFunctionType.Exp

    nc = bacc.Bacc(
        "TRN2", target_bir_lowering=False, debug=False, num_devices=NCORES
    )

    maskT = nc.dram_tensor("maskT", [S, QC], bf16, kind="ExternalInput").ap()
    keysT = nc.dram_tensor("keysT", [E, S], bf16, kind="ExternalInput").ap()
    valsT = nc.dram_tensor("valsT", [E, S], bf16, kind="ExternalInput").ap()
    qryT = nc.dram_tensor("qryT", [E, QC], bf16, kind="ExternalInput").ap()
    wqT = nc.dram_tensor("wqT", [E, E], bf16, kind="ExternalInput").ap()
    wkT = nc.dram_tensor("wkT", [E, E], bf16, kind="ExternalInput").ap()
    wvT = nc.dram_tensor("wvT", [E, E], bf16, kind="ExternalInput").ap()
    wo2d = nc.dram_tensor("wo2d", [P, NPAIR, E], bf16, kind="ExternalInput").ap()
    bo = nc.dram_tensor("bo", [E], f32, kind="ExternalInput").ap()
    out = nc.dram_tensor("out", [QC, E], f32, kind="ExternalOutput").ap()

    with tile.TileContext(nc) as tc:
        with tc.tile_pool(name="persist", bufs=1) as persist:
            # persistent SBUF tensors (per-partition bytes in comments)
            maskb = persist.tile([P, NKT, QC], bf16)         # 64 KB
            v_all = persist.tile([P, NKT, H, HD + 1], bf16)  # 33.3 KB
            kT_all = persist.tile([P, NPAIR, S], bf16)       # 32 KB
            qT_all = persist.tile([P, NPAIR, QC], bf16)      # 8 KB
            attn2 = persist.tile([P, NPAIR, QC], bf16)       # 8 KB
            wq_sb = persist.tile([P, 4, E], bf16)            # 4 KB
            wk_sb = persist.tile([P, 4, E], bf16)            # 4 KB
            wv_sb = persist.tile([P, 4, E], bf16)            # 4 KB
            wo_sb = persist.tile([P, NPAIR, E], bf16)        # 4 KB
            qry_sb = persist.tile([P, 4, QC], bf16)          # 8 KB
            bo_sb = persist.tile([1, E], f32)

            nc.sync.dma_start(
                out=wq_sb, in_=wqT.rearrange("(g p) o -> p g o", p=P)
            )
            nc.sync.dma_start(
                out=wk_sb, in_=wkT.rearrange("(g p) o -> p g o", p=P)
            )
            nc.sync.dma_start(
                out=wv_sb, in_=wvT.rearrange("(g p) o -> p g o", p=P)
            )
            nc.sync.dma_start(out=wo_sb, in_=wo2d)
            nc.sync.dma_start(out=bo_sb, in_=bo[None, :])
            nc.sync.dma_start(
                out=qry_sb, in_=qryT.rearrange("(g p) q -> p g q", p=P)
            )

            # mask: bf16 0/1, [k-part, kt, q]
            for kt in range(NKT):
                nc.sync.dma_start(
                    out=maskb[:, kt, :],
                    in_=maskT[kt * P : (kt + 1) * P, :],
                )

            # ones column for the softmax denominator
            nc.vector.memset(v_all[:, :, :, HD : HD + 1], 1.0)

            # ---- Q projection (up front, cheap: 16k columns) ----
            with tc.tile_pool(name="qps", bufs=2, space="PSUM") as qps:
                for qt in range(NQT):
                    qsl = slice(qt * 512, (qt + 1) * 512)
                    for c in range(NPAIR):
                        ps = qps.tile([P, 512], f32)
                        for ec in range(4):
                            nc.tensor.matmul(
                                ps,
                                lhsT=wq_sb[:, ec, c * P : (c + 1) * P],
                                rhs=qry_sb[:, ec, qsl],
                                start=(ec == 0),
                                stop=(ec == 3),
                            )
                        nc.vector.tensor_copy(out=qT_all[:, c, qsl], in_=ps)

            # ---- attention (K proj per pair, V proj inside c0/qt0) ----
            with (
                tc.tile_pool(name="kstage", bufs=2) as kstage,
                tc.tile_pool(name="vstage", bufs=2) as vstage,
                tc.tile_pool(name="kps", bufs=1, space="PSUM") as kps,
                tc.tile_pool(name="vps", bufs=1, space="PSUM") as vps,
                tc.tile_pool(name="scps", bufs=2, space="PSUM") as scps,
                tc.tile_pool(name="pvps", bufs=2, space="PSUM") as pvps,
                tc.tile_pool(name="pp", bufs=4) as pp,
                tc.tile_pool(name="norm", bufs=2) as norm,
                tc.tile_pool(name="ndram", bufs=2, space="DRAM") as ndram,
            ):
                def kproj_chunk(cc, kc):
                    # one 512-key chunk of pair cc's K projection
                    ksl = slice(kc * 512, (kc + 1) * 512)
                    ks = kstage.tile([P, 4, 512], bf16, name="ks")
                    nc.sync.dma_start(
                        out=ks,
                        in_=keysT[:, ksl].rearrange("(g p) s -> p g s", p=P),
                    )
                    ps = kps.tile([P, 512], f32, name="kp")
                    for ec in range(4):
                        nc.tensor.matmul(
                            ps,
                            lhsT=wk_sb[:, ec, cc * P : (cc + 1) * P],
                            rhs=ks[:, ec, :],
                            start=(ec == 0),
                            stop=(ec == 3),
                        )
                    nc.vector.tensor_copy(out=kT_all[:, cc, ksl], in_=ps)

                for kc in range(S // 512):
                    kproj_chunk(0, kc)
                for c in range(NPAIR):
                    for qt in range(NQT):
                        qsl = slice(qt * 512, (qt + 1) * 512)
                        pv0 = pvps.tile([HD + 1, 512], f32, tag="pv")
                        pv1 = pvps.tile([HD + 1, 512], f32, tag="pv")
                        for kt in range(NKT):
                            if c == 0 and qt == 0:
                                # V projection for this kt
                                vs = vstage.tile([P, 4, P], bf16)
                                nc.sync.dma_start(
                                    out=vs,
                                    in_=valsT[:, kt * P : (kt + 1) * P]
                                    .rearrange("(g p) s -> p g s", p=P),
                                )
                                vp = vps.tile([P, E], f32)
                                for ec in range(4):
                                    nc.tensor.matmul(
                                        vp,
                                        lhsT=vs[:, ec, :],
                                        rhs=wv_sb[:, ec, :],
                                        start=(ec == 0),
                                        stop=(ec == 3),
                                    )
                                nc.vector.tensor_copy(
                                    out=v_all[:, kt, :, 0:HD],
                                    in_=vp.rearrange("p (h d) -> p h d", h=H),
                                )
                            if (
                                qt == NQT - 1
                                and kt % 4 == 0
                                and c + 1 < NPAIR
                            ):
                                kproj_chunk(c + 1, kt // 4)
                            ksl = slice(kt * P, (kt + 1) * P)
                            sc = scps.tile([P, 2, 512], f32, tag="sc")
                            nc.tensor.matmul(
                                sc[:, 0, :],
                                lhsT=kT_all[0:HD, c, ksl],
                                rhs=qT_all[0:HD, c, qsl],
                                start=True,
                                stop=True,
                            )
                            nc.tensor.matmul(
                                sc[:, 1, :],
                                lhsT=kT_all[HD : 2 * HD, c, ksl],
                                rhs=qT_all[HD : 2 * HD, c, qsl],
                                start=True,
                                stop=True,
                            )
                            p_sb = pp.tile([P, 2, 512], bf16)
                            nc.scalar.activation(p_sb, sc, Exp, scale=SCALE)
                            meng = nc.gpsimd if kt in POOL_KT else nc.vector
                            for s_ in range(2):
                                meng.tensor_tensor(
                                    out=p_sb[:, s_, :],
                                    in0=p_sb[:, s_, :],
                                    in1=maskb[:, kt, qsl],
                                    op=mybir.AluOpType.mult,
                                )
                            nc.tensor.matmul(
                                pv0,
                                lhsT=v_all[:, kt, 2 * c, :],
                                rhs=p_sb[:, 0, :],
                                start=(kt == 0),
                                stop=(kt == NKT - 1),
                            )
                            nc.tensor.matmul(
                                pv1,
                                lhsT=v_all[:, kt, 2 * c + 1, :],
                                rhs=p_sb[:, 1, :],
                                start=(kt == 0),
                                stop=(kt == NKT - 1),
                            )
                        for s_, pv in ((0, pv0), (1, pv1)):
                            # copy PV out of PSUM right away (frees the bank
                            # for the next (c, qt) iteration's accumulation)
                            pv_sb = norm.tile([P, 512], f32, tag="den")
                            nc.vector.tensor_copy(
                                out=pv_sb[0 : HD + 1, :],
                                in_=pv[0 : HD + 1, :],
                            )
                            # replicate den across partitions 0..63 via a
                            # DRAM bounce (DRAM sources allow stride-0
                            # partition broadcast APs; SBUF sources don't),
                            # then reciprocal at base partition 0
                            dscr = ndram.tile([1, 512], f32, tag="dscr")
                            nc.sync.dma_start(
                                out=dscr, in_=pv_sb[HD : HD + 1, :]
                            )
                            den_rep = norm.tile([HD, 512], f32, tag="denr")
                            nc.sync.dma_start(
                                out=den_rep,
                                in_=bass.AP(
                                    tensor=dscr.tensor,
                                    offset=dscr.offset,
                                    ap=[[0, HD], [1, 512]],
                                ),
                            )
                            rep_sb = norm.tile([HD, 512], f32, tag="rep")
                            nc.vector.reciprocal_approx_fast(
                                out=rep_sb, in_=den_rep
                            )
                            if s_ == 0:
                                nc.vector.tensor_tensor(
                                    out=attn2[0:HD, c, qsl],
                                    in0=pv_sb[0:HD, :],
                                    in1=rep_sb,
                                    op=mybir.AluOpType.mult,
                                )
                            else:
                                # odd head: normalize into a temp, then
                                # DMA-shift to partitions 64..127 so the
                                # output projection can pack the pair
                                # (contract 128)
                                atmp = norm.tile([HD, 512], bf16, tag="atm")
                                nc.vector.tensor_tensor(
                                    out=atmp,
                                    in0=pv_sb[0:HD, :],
                                    in1=rep_sb,
                                    op=mybir.AluOpType.mult,
                                )
                                nc.sync.dma_start(
                                    out=attn2[HD : 2 * HD, c, qsl],
                                    in_=atmp,
                                )

            # ---- output projection (2 heads packed) + bias ----
            with (
                tc.tile_pool(name="ops", bufs=2, space="PSUM") as ops,
                tc.tile_pool(name="osb", bufs=3) as osb,
                tc.tile_pool(name="onesp", bufs=1) as onesp,
            ):
                ones1 = onesp.tile([1, P], f32)
                nc.vector.memset(ones1, 1.0)
                for q8 in range(QC // P):
                    ps = ops.tile([P, E], f32)
                    for c in range(NPAIR):
                        nc.tensor.matmul(
                            ps,
                            lhsT=attn2[:, c, q8 * P : (q8 + 1) * P],
                            rhs=wo_sb[:, c, :],
                            start=(c == 0),
                            stop=False,
                        )
                    # bias via rank-1 matmul: ones^T (1x128) @ bo (1x512)
                    nc.tensor.matmul(
                        ps,
                        lhsT=ones1,
                        rhs=bo_sb,
                        start=False,
                        stop=True,
                    )
                    ob = osb.tile([P, E], f32)
                    nc.vector.tensor_copy(out=ob, in_=ps)
                    nc.sync.dma_start(
                        out=out[q8 * P : (q8 + 1) * P, :], in_=ob
                    )

    nc.compile()
    _CACHE["nc"] = nc
    return nc


def make_in_maps(values, keys, query, mask, Wv, Wk, Wq, Wo, bo):
    values = np.asarray(values, np.float32)
    keys = np.asarray(keys, np.float32)
    query = np.asarray(query, np.float32)
    mask = np.asarray(mask)
    wqT = np.ascontiguousarray(np.asarray(Wq, np.float32).T.astype(BF16))
    wkT = np.ascontiguousarray(np.asarray(Wk, np.float32).T.astype(BF16))
    wvT = np.ascontiguousarray(np.asarray(Wv, np.float32).T.astype(BF16))
    # wo2d[s*64+d, c, e] = Wo[e, (2c+s)*64+d]
    wo2d = np.ascontiguousarray(
        np.asarray(Wo, np.float32).T.reshape(NPAIR, 2, HD, E)
        .transpose(1, 2, 0, 3).reshape(P, NPAIR, E).astype(BF16)
    )
    bo = np.ascontiguousarray(np.asarray(bo, np.float32))

    in_maps = []
    for core in range(NCORES):
        b, qc = core // (NCORES // B), core % (NCORES // B)
        qsl = slice(qc * QC, (qc + 1) * QC)
        in_maps.append(
            {
                "maskT": np.ascontiguousarray(
                    mask[b, 0, qsl, :].T.astype(BF16)
                ),
                "keysT": np.ascontiguousarray(keys[b].T.astype(BF16)),
                "valsT": np.ascontiguousarray(values[b].T.astype(BF16)),
                "qryT": np.ascontiguousarray(query[b, qsl].T.astype(BF16)),
                "wqT": wqT,
                "wkT": wkT,
                "wvT": wvT,
                "wo2d": wo2d,
                "bo": bo,
            }
        )
    return in_maps


def kernel(values, keys, query, mask, Wv, Wk, Wq, Wo, bo):
    global LAST_RESULT
    from concourse.bass_utils import run_bass_kernel_spmd

    nc = _build()
    in_maps = make_in_maps(values, keys, query, mask, Wv, Wk, Wq, Wo, bo)
    res = run_bass_kernel_spmd(nc, in_maps, core_ids=list(range(NCORES)))
    LAST_RESULT = res

    out = np.empty((B, S, E), np.float32)
    for core in range(NCORES):
        b, qc = core // (NCORES // B), core % (NCORES // B)
        out[b, qc * QC : (qc + 1) * QC] = res.results[core]["out"]
    return out
